# revision 3
# baseline (speedup 1.0000x reference)
"""MDTA (Restormer multi-dconv-head transposed attention) Trainium2 kernel, v2.

Distribution: data-parallel over batch B=8 across 8 NeuronCores (one image
per core, weights replicated, no collectives).

Per-core pipeline (image = 192ch x 128x128, fp32 in/out):
  1. x loaded fp16 via gpsimd cast-DMA
  2. 1x1 qkv conv     : PE matmul fp16 -> psum -> u fp16 [128,5,18,132]
                        chunks {q128, q64|k64, k128, v96, v96}
  3. depthwise 3x3    : q,k chunks on DVE (6 aligned taps 2x-mode) with ACT
                        computing the 3 center-column taps into tmp + DVE add;
                        v chunks on PE (diag matmuls) -> v_sb [96,2,N] fp16
  4. q,k              : spilled to DRAM fp16, read back via xbar DMA-transpose
                        [384,128]->[128,384], split across sync+scalar rings
     gram S_h=q_h k_h^T: per-head 48x48, PSUM-accumulated over all 16384 px,
                        pipelined one tile behind the transposes
     norms            : ACT Square with accum_out per tile
  5. attn finalize    : scale by 1/||q||/||k||*temp, softmax -> attn_g [96,96]
     M^T = attn^T @ Wp^T (2 matmuls) -> y = M^T.T @ v : PE, PSUM -> DRAM direct
"""

import os
import numpy as np

# Hardcoded problem shape (nn_MDTA_74045236183622)
B = 8
C = 192
C3 = 3 * C  # 576
H = W = 128
NPIX = H * W  # 16384
NH, DH = 4, 48
EPS = 1e-12

ROWS = 16                 # output rows per spatial tile
NT = H // ROWS            # 8 tiles
TPX = ROWS * W            # 2048 px per tile
WP = W + 4                # x-padded row width in u (132 -> 264B rows, 4B align)

# u channel chunks over the 576 qkv channels
CHUNK_OFF = [0, 128, 256, 384, 480]
CHUNK_W = [128, 128, 128, 96, 96]
# dw-conv rows owned by PE per chunk (from row 0); the rest go to DVE
DW_PE_ROWS = [int(v) for v in os.environ.get(
    "MDTA_DW_PE", "0,0,8,16,16").split(",")]
REPEAT = 1
PHASES = int(os.environ.get("MDTA_PHASES", "4"))  # 1=qkv 2=+dw 3=+gram 4=full

# taps ordered: aligned (dx=+-1) first, center column (dx=0) last
TAPS_ALIGNED = [(dy, dx) for dy in (-1, 0, 1) for dx in (-1, 1)]
TAPS_CENTER = [(dy, 0) for dy in (-1, 0, 1)]


def build_kernel(repeat=None):
    import concourse.bass as bass
    import concourse.tile as tile
    from concourse import bacc, mybir
    from concourse.masks import make_identity

    f32 = mybir.dt.float32
    f16 = mybir.dt.float16

    nc = bacc.Bacc("TRN2", target_bir_lowering=False, debug=False,
                   enable_asserts=False, num_devices=1)

    x_d = nc.dram_tensor("x", (C, H, W), f32, kind="ExternalInput").ap()
    wqkv_d = nc.dram_tensor("w_qkv", (C3, C), f32, kind="ExternalInput").ap()
    wdw_d = nc.dram_tensor("w_dw", (C3, 1, 3, 3), f32, kind="ExternalInput").ap()
    wproj_d = nc.dram_tensor("w_proj", (C, C), f32, kind="ExternalInput").ap()
    temp_d = nc.dram_tensor("temperature", (NH, 1, 1), f32, kind="ExternalInput").ap()
    out_d = nc.dram_tensor("out", (C, H, W), f32, kind="ExternalOutput").ap()

    with tile.TileContext(nc) as tc:
        _emit(tc, bass, mybir, make_identity, f32, f16,
              x_d, wqkv_d, wdw_d, wproj_d, temp_d, out_d,
              repeat if repeat is not None else REPEAT)
    nc.compile()
    return nc


def _emit(tc, bass, mybir, make_identity, f32, f16,
          x_d, wqkv_d, wdw_d, wproj_d, temp_d, out_d, repeat):
    from contextlib import ExitStack
    ctx = ExitStack()
    nc = tc.nc
    Alu = mybir.AluOpType
    Act = mybir.ActivationFunctionType

    persist = ctx.enter_context(tc.tile_pool(name="persist", bufs=1))
    xpool = ctx.enter_context(tc.tile_pool(name="xpool", bufs=2))
    upool = ctx.enter_context(tc.tile_pool(name="upool", bufs=1))
    stpool = ctx.enter_context(tc.tile_pool(name="stage", bufs=2))
    qkpool = ctx.enter_context(tc.tile_pool(name="qkT", bufs=2))
    tmpool = ctx.enter_context(tc.tile_pool(name="tmul", bufs=8))
    vstpool = ctx.enter_context(tc.tile_pool(name="vstage", bufs=1))
    vldpool = ctx.enter_context(tc.tile_pool(name="vload", bufs=4))
    dram = ctx.enter_context(tc.tile_pool(name="dram", bufs=1, space="DRAM"))
    psA = ctx.enter_context(tc.tile_pool(name="psA", bufs=2, space="PSUM"))
    psB = ctx.enter_context(tc.tile_pool(name="psB", bufs=2, space="PSUM"))
    psG = ctx.enter_context(tc.tile_pool(name="psG", bufs=1, space="PSUM"))

    # ---------------- setup: weights into SBUF ----------------
    ident128h = persist.tile([128, 128], f16)
    make_identity(nc, ident128h)
    ident96h = persist.tile([96, 96], f16)
    make_identity(nc, ident96h)

    # natural (contiguous) weight loads, cast fp16, then on-chip PE transposes
    wq_nat = xpool.tile([128, 5, C], f32, name="wq_nat", tag="xh")
    nc.sync.dma_start(wq_nat[:, 0:4, :],
                      wqkv_d[0:512].rearrange("(ci p) c -> p ci c", p=128))
    nc.sync.dma_start(wq_nat[0:64, 4, :], wqkv_d[512:576])
    wq_h = stpool.tile([128, 5, C], f16, name="wq_h", tag="stage")
    nc.vector.tensor_copy(out=wq_h[:, 0:4, :], in_=wq_nat[:, 0:4, :])
    nc.vector.tensor_copy(out=wq_h[0:64, 4, :], in_=wq_nat[0:64, 4, :])

    # w_qkv^T as lhsT: [c_part, o]; K split 128+64  (o chunking done at use)
    wqkvT_a = persist.tile([128, C3], f16)
    wqkvT_b = persist.tile([64, C3], f16)
    for ci in range(5):
        m = 128 if ci < 4 else 64
        o0 = ci * 128
        for kc, kw in ((0, 128), (1, 64)):
            wtp = psA.tile([128, 512], f16, tag="psA", name="wtp")
            nc.tensor.transpose(wtp[0:kw, 0:m],
                                wq_h[0:m, ci, kc * 128:kc * 128 + kw],
                                ident128h[0:m, 0:m])
            dst = wqkvT_a if kc == 0 else wqkvT_b
            nc.scalar.copy(dst[0:kw, o0:o0 + m], wtp[0:kw, 0:m])

    # depthwise weights: wdw_sb[p, ci, t] = w_dw[CHUNK_OFF[ci]+p, 0, t//3, t%3]
    wdw_sb = persist.tile([128, 5, 9], f32)
    wdw_flat = wdw_d.rearrange("o one ky kx -> o (one ky kx)")  # (576, 9)
    with nc.allow_non_contiguous_dma(reason="one-time dw weight load"):
        nc.sync.dma_start(
            wdw_sb[:, 0:3, :],
            wdw_flat[0:384].rearrange("(ci p) t -> p ci t", p=128))
        nc.sync.dma_start(wdw_sb[0:96, 3, :], wdw_flat[384:480])
        nc.sync.dma_start(wdw_sb[0:96, 4, :], wdw_flat[480:576])

    # diag(w_dw) blocks for the PE depthwise path, fp16
    pe_chunks = [ci for ci in range(5) if DW_PE_ROWS[ci] > 0]
    diag_sb = persist.tile([128, 9, len(pe_chunks), 128], f16)
    for t in range(9):
        for k, ci in enumerate(pe_chunks):
            m = CHUNK_W[ci]
            nc.vector.tensor_scalar_mul(
                diag_sb[0:m, t, k, 0:m], ident128h[0:m, 0:m],
                wdw_sb[0:m, ci, t:t + 1])
    diag_idx = {ci: k for k, ci in enumerate(pe_chunks)}

    # w_proj^T as lhsT: [d_part(96), g, o] fp16
    wp_nat = xpool.tile([96, 2, C], f32, name="wp_nat", tag="xh")
    nc.sync.dma_start(wp_nat, wproj_d.rearrange("(ko p) c -> p ko c", p=96))
    wp_h = stpool.tile([96, 2, C], f16, name="wp_h", tag="stage")
    nc.vector.tensor_copy(out=wp_h, in_=wp_nat)
    wpT = persist.tile([96, 2, C], f16)
    for ko in range(2):
        for kc in range(2):
            wtp2 = psA.tile([96, 96], f16, tag="psA", name="wtp2")
            nc.tensor.transpose(wtp2, wp_h[:, ko, kc * 96:kc * 96 + 96],
                                ident96h)
            nc.scalar.copy(wpT[:, kc, ko * 96:ko * 96 + 96], wtp2)

    # persistent working buffers
    u_t = [upool.tile([128, 5, ROWS + 2, WP], f16, name=f"u{j}")
           for j in range(2)]
    for j in range(2):
        nc.vector.memset(u_t[j][:, :, :, 0:1], 0.0)      # left pad col
        nc.vector.memset(u_t[j][:, :, :, W + 1:WP], 0.0)  # right pad cols
    np_part = persist.tile([128, 3, NT], f32)             # per-tile sum-of-squares
    v_dram = dram.tile([2, 96, NPIX], f16)                # v spill
    qk_dram = dram.tile([384, NPIX], f16)                 # q,k spill for transposes
    qk_w = qk_dram.rearrange("(ci p) n -> p ci n", p=128)

    Gps = psG.tile([96, 2, 96], f32, tag="psG", name="Gps")  # gram accum (2-head packed)

    TAPS9 = TAPS_ALIGNED + TAPS_CENTER

    # ---------------- pass 1: one spatial tile ----------------
    def pass1_tile(i, do_gram):
        y0 = i * ROWS
        u = u_t[i % 2]

        # x rows y0-1 .. y0+16 -> xh rows 0..17 (fp16 cast in DMA)
        xh = xpool.tile([128, 2, ROWS + 2, W], f16, name="xh")
        lo = max(y0 - 1, 0)
        hi = min(y0 + ROWS + 1, H)
        ur0 = lo - (y0 - 1)
        if i == 0:
            nc.vector.memset(xh[:, :, 0:1, :], 0.0)
        if i == NT - 1:
            nc.vector.memset(xh[:, :, ROWS + 1:ROWS + 2, :], 0.0)
        nc.gpsimd.dma_start(xh[:, 0, ur0:ur0 + (hi - lo), :], x_d[0:128, lo:hi, :])
        nc.gpsimd.dma_start(xh[0:64, 1, ur0:ur0 + (hi - lo), :],
                            x_d[128:192, lo:hi, :])

        # ---- 1x1 qkv conv: u rows 0..17 ----
        rgroups = [(0, 4), (4, 8), (8, 12), (12, 16), (16, 18)]
        for gi, (r0, r1) in enumerate(rgroups):
            n = (r1 - r0) * W
            for ci in range(5):
                m = CHUNK_W[ci]
                o0 = CHUNK_OFF[ci]
                ps = psA.tile([128, 512], f32, tag="psA", name="ups")
                nc.tensor.matmul(
                    ps[0:m, 0:n], lhsT=wqkvT_a[:, o0:o0 + m],
                    rhs=xh[:, 0, r0:r1, :], start=True, stop=False)
                nc.tensor.matmul(
                    ps[0:m, 0:n], lhsT=wqkvT_b[:, o0:o0 + m],
                    rhs=xh[0:64, 1, r0:r1, :], start=False, stop=True)
                udst = u[0:m, ci, r0:r1, 1:W + 1]
                usrc = ps[0:m, 0:n].rearrange("p (r c) -> p r c", c=W)
                if (gi * 5 + ci) % 5 < 3:
                    nc.scalar.copy(udst, usrc)
                else:
                    nc.vector.tensor_copy(out=udst, in_=usrc)

        # ---- gram for previous tile (transposes have had a tile to land) ----
        if do_gram is not None:
            qkT_p, ip = do_gram
            for blk in range(16):
                for g in range(2):
                    nc.tensor.matmul(
                        Gps[:, g, :],
                        lhsT=qkT_p[:, blk, g * 96:g * 96 + 96],
                        rhs=qkT_p[:, blk, C + g * 96:C + g * 96 + 96],
                        start=(ip == 0 and blk == 0),
                        stop=(ip == NT - 1 and blk == 15),
                        skip_group_check=True)

        if PHASES < 2:
            return None
        # ---- depthwise 3x3 ----
        stage = stpool.tile([128, 3, TPX], f16, name="stage")
        vst = vstpool.tile([96, 2, TPX], f16, name="vst")

        def dw_pe(ci, ov, m, r0, r1):
            k = diag_idx[ci]
            for oy in range(r0, r1, 4):  # out rows oy..oy+4
                ps = psB.tile([128, 512], f32, tag="psB", name="dps")
                for t, (dy, dx) in enumerate(TAPS9):
                    tcol = 3 * (dy + 1) + dx + 1
                    nc.tensor.matmul(
                        ps[0:m, :],
                        lhsT=diag_sb[0:m, tcol, k, 0:m],
                        rhs=u[0:m, ci, oy + dy + 1:oy + dy + 5,
                              dx + 1:dx + 1 + W],
                        start=(t == 0), stop=(t == 8))
                dst = ov[:, oy:oy + 4, :]
                src = ps[0:m, :].rearrange("p (r c) -> p r c", c=W)
                if (oy // 4 + ci) % 2 == 0:
                    nc.scalar.copy(dst, src)
                else:
                    nc.vector.tensor_copy(out=dst, in_=src)

        def dw_dve(ci, ov, m, r0, r1):
            # DVE: 5 aligned taps (TS 4x mults + TT 2x adds)
            # Pool: 1 aligned tap + add of one ACT tmp -> pacc
            # ACT: 3 center-column tap mults -> tmps
            rows = r1 - r0
            wcol = wdw_sb[0:m, ci, :]
            ovs = ov[:, r0:r1, :]

            def shift(dy, dx):
                return u[0:m, ci, r0 + dy + 1:r0 + dy + 1 + rows,
                         dx + 1:dx + 1 + W]

            def wc(dy, dx):
                tcol = 3 * (dy + 1) + dx + 1
                return wcol[:, tcol:tcol + 1]

            cm0 = tmpool.tile([128, ROWS, W], f16, name="tm", tag="tm")
            cm1 = tmpool.tile([128, ROWS, W], f16, name="tm", tag="tm")
            cm2 = tmpool.tile([128, ROWS, W], f16, name="tm", tag="tm")
            for tmv, (dy, dx) in zip((cm0, cm1, cm2), TAPS_CENTER):
                nc.scalar.activation(tmv[0:m, 0:rows], shift(dy, dx), Act.Copy,
                                     scale=wc(dy, dx))
            # Pool subtree
            pacc = tmpool.tile([128, ROWS, W], f16, name="pacc", tag="tm")
            dyp, dxp = TAPS_ALIGNED[5]
            nc.gpsimd.tensor_scalar_mul(pacc[0:m, 0:rows], shift(dyp, dxp),
                                        wc(dyp, dxp))
            nc.gpsimd.tensor_tensor(pacc[0:m, 0:rows], pacc[0:m, 0:rows],
                                    cm2[0:m, 0:rows], op=Alu.add)
            # DVE main chain
            dy0, dx0 = TAPS_ALIGNED[0]
            nc.vector.tensor_scalar_mul(ovs, shift(dy0, dx0), wc(dy0, dx0))
            for (dy, dx) in TAPS_ALIGNED[1:5]:
                tv = tmpool.tile([128, ROWS, W], f16, name="tv", tag="tm")
                nc.vector.tensor_scalar_mul(tv[0:m, 0:rows], shift(dy, dx),
                                            wc(dy, dx))
                nc.vector.tensor_tensor(ovs, ovs, tv[0:m, 0:rows], op=Alu.add)
            nc.vector.tensor_tensor(ovs, ovs, cm0[0:m, 0:rows], op=Alu.add)
            nc.vector.tensor_tensor(ovs, ovs, cm1[0:m, 0:rows], op=Alu.add)
            nc.vector.tensor_tensor(ovs, ovs, pacc[0:m, 0:rows], op=Alu.add)

        sl = slice(i * TPX, (i + 1) * TPX)
        for ci in range(3):
            m = CHUNK_W[ci]
            ov = stage[0:m, ci, :].rearrange("p (r c) -> p r c", c=W)
            rpe = DW_PE_ROWS[ci]
            if rpe > 0:
                dw_pe(ci, ov, m, 0, rpe)
            if rpe < ROWS:
                dw_dve(ci, ov, m, rpe, ROWS)

        if PHASES >= 3:
            # ---- q,k -> DRAM + transpose readbacks, before the v chunks ----
            nc.sync.dma_start(qk_w[:, 0:2, sl], stage[:, 0:2, :])
            nc.scalar.dma_start(qk_w[:, 2:3, sl], stage[:, 2:3, :])
            qkT = qkpool.tile([128, 16, 384], f16, name="qkT")
            for blk in range(16):
                n0 = i * TPX + blk * 128
                eng = nc.sync if blk % 2 == 0 else nc.scalar
                eng.dma_start_transpose(qkT[:, blk, :], qk_dram[:, n0:n0 + 128])

        for ci in range(3, 5):
            m = CHUNK_W[ci]
            ov = vst[0:m, ci - 3, :].rearrange("p (r c) -> p r c", c=W)
            rpe = DW_PE_ROWS[ci]
            if rpe > 0:
                dw_pe(ci, ov, m, 0, rpe)
            if rpe < ROWS:
                dw_dve(ci, ov, m, rpe, ROWS)

        if PHASES < 3:
            return None
        # ---- norms partial: ACT square with accumulate ----
        # (in-place square after the spill DMAs have read the stage)
        for ci in range(3):
            nc.scalar.activation(
                stage[:, ci, :], stage[:, ci, :], Act.Square,
                accum_out=np_part[:, ci, i:i + 1])

        # ---- v -> DRAM ----
        nc.gpsimd.dma_start(v_dram[:, :, sl].rearrange("g p n -> p g n"), vst)
        return qkT

    # ---------------- pass 2: finalize attention + output ----------------
    def pass2_normprep():
        rn = persist.tile([128, 3], f32, name="rn")
        nc.vector.tensor_reduce(rn, np_part, axis=mybir.AxisListType.X,
                                op=Alu.add)
        nc.scalar.sqrt(rn, rn)
        nc.vector.tensor_scalar_max(rn, rn, EPS)
        nc.vector.reciprocal(rn, rn)

        nrm_dram = dram.tile([128, 3], f32, name="nrm_dram")
        nc.sync.dma_start(nrm_dram, rn)
        # rnq_e/rnq_o[p, g] = 1/||q|| for heads 2g / 2g+1 (q ch c -> rn[c%128, c//128])
        rnq_e = persist.tile([48, 2], f32, name="rnq_e")
        rnq_o = persist.tile([48, 2], f32, name="rnq_o")
        nc.sync.dma_start(rnq_e[:, 0:1], nrm_dram[0:48, 0:1])
        nc.sync.dma_start(rnq_e[0:32, 1:2], nrm_dram[96:128, 0:1])
        nc.sync.dma_start(rnq_e[32:48, 1:2], nrm_dram[0:16, 1:2])
        nc.sync.dma_start(rnq_o[:, 0:1], nrm_dram[48:96, 0:1])
        nc.sync.dma_start(rnq_o[:, 1:2], nrm_dram[16:64, 1:2])
        # temperature: tg_e[p, g] = temp[2g], tg_o[p, g] = temp[2g+1]
        tg = persist.tile([48, 2, 2], f32, name="tg")
        nc.gpsimd.dma_start(
            tg[:, 0, :], bass.AP(tensor=temp_d.tensor, offset=temp_d.offset,
                                 ap=[[0, 48], [2, 2]]))
        nc.gpsimd.dma_start(
            tg[:, 1, :], bass.AP(tensor=temp_d.tensor,
                                 offset=temp_d.offset + 1,
                                 ap=[[0, 48], [2, 2]]))
        nc.vector.tensor_mul(rnq_e, rnq_e, tg[:, 0, :])
        nc.vector.tensor_mul(rnq_o, rnq_o, tg[:, 1, :])

        rnk_row = persist.tile([1, 192], f32, name="rnk_row")
        with nc.allow_non_contiguous_dma(reason="tiny norm vector transpose"):
            nc.sync.dma_start(rnk_row[0:1, 0:64],
                              nrm_dram[64:128, 1:2].rearrange("p o -> o p"))
            nc.sync.dma_start(rnk_row[0:1, 64:192],
                              nrm_dram[0:128, 2:3].rearrange("p o -> o p"))
        ones_row = persist.tile([1, 96], f32, name="ones_row")
        nc.vector.memset(ones_row, 1.0)
        rnk_bc = persist.tile([96, 2, 96], f32, name="rnk_bc")
        for g in range(2):
            bc_ps = psA.tile([128, 512], f32, tag="psA", name="bc_ps")
            nc.tensor.matmul(bc_ps[0:96, 0:96], lhsT=ones_row,
                             rhs=rnk_row[0:1, g * 96:g * 96 + 96],
                             start=True, stop=True)
            nc.vector.tensor_copy(out=rnk_bc[:, g, :], in_=bc_ps[0:96, 0:96])
        return (rnq_e, rnq_o), rnk_bc

    def pass2_attn(rnq8, rnk_bc):
        rnq_e, rnq_o = rnq8
        # scale + softmax per head, assemble block-diag attn groups [96, 96]
        # (odd heads sit at partition offset 48 in the packed gram: realign
        #  their 48x48 blocks to partition 0 via SBUF-SBUF DMA, softmax at
        #  base 0, then DMA the result back to offset 48)
        Sg = persist.tile([96, 2, 96], f32, name="Sg")
        nc.vector.tensor_copy(out=Sg, in_=Gps)
        So = persist.tile([48, 2, DH], f32, name="So")
        for g in range(2):
            nc.sync.dma_start(So[:, g, :], Sg[48:96, g, 48:96])
        attn_g = [persist.tile([96, 96], f16, name=f"attn_g{g}")
                  for g in range(2)]
        for g in range(2):
            nc.vector.memset(attn_g[g], 0.0)
        mx = persist.tile([48, 1], f32, name="mx")
        sm = persist.tile([48, 1], f32, name="sm")
        at16 = persist.tile([48, DH], f16, name="at16")
        for h in range(NH):
            g, odd = h // 2, h % 2
            r0 = odd * DH
            blkS = So[:, g, :] if odd else Sg[0:DH, g, 0:DH]
            rnq = rnq_o if odd else rnq_e
            nc.vector.scalar_tensor_tensor(
                blkS, blkS, rnq[:, g:g + 1],
                rnk_bc[0:DH, g, r0:r0 + DH],
                op0=Alu.mult, op1=Alu.mult)
            nc.vector.tensor_reduce(mx, blkS, axis=mybir.AxisListType.X,
                                    op=Alu.max, negate=True)
            nc.scalar.activation(blkS, blkS, Act.Exp, bias=mx, scale=1.0)
            nc.vector.tensor_reduce(sm, blkS, axis=mybir.AxisListType.X,
                                    op=Alu.add)
            nc.vector.reciprocal(sm, sm)
            if odd:
                nc.vector.tensor_scalar_mul(at16, blkS, sm)
                nc.scalar.dma_start(attn_g[g][48:96, 48:96], at16)
            else:
                nc.vector.tensor_scalar_mul(attn_g[g][0:DH, 0:DH], blkS, sm)

        # M^T[e, o] = sum_d attn[d, e] WpT[d, o]  (block-diag per 96-group)
        MT = persist.tile([96, 2, C], f16, name="MT")
        for g in range(2):
            mps = psA.tile([128, 512], f32, tag="psA", name="mps")
            nc.tensor.matmul(mps[0:96, 0:C], lhsT=attn_g[g], rhs=wpT[:, g, :],
                             start=True, stop=True)
            nc.vector.tensor_copy(out=MT[:, g, :], in_=mps[0:96, 0:C])

        return MT

    def load_v(pg):
        px = pg * 1024
        vld = vldpool.tile([96, 2, 1024], f16, name="vld", tag="vld")
        nc.gpsimd.dma_start(
            vld, v_dram[:, :, px:px + 1024].rearrange("g p n -> p g n"))
        return vld

    def pass2_y(MT, vlds):
        # y = M @ v : PSUM -> SBUF -> DRAM (1024-px groups)
        o_flat = out_d.rearrange("c h w -> c (h w)")
        npg = NPIX // 1024
        for pg in range(npg):
            px = pg * 1024
            vld = vlds[pg]
            if pg + 2 < npg:
                vlds.append(load_v(pg + 2))
            for oc, (obase, ow) in enumerate(((0, 128), (128, 64))):
                y_sb = vldpool.tile([128, 1024], f32, name="y_sb", tag="vld")
                for half in range(2):
                    yps = psA.tile([128, 512], f32, tag="psA", name="yps")
                    for g in range(2):
                        nc.tensor.matmul(
                            yps[0:ow, :], lhsT=MT[:, g, obase:obase + ow],
                            rhs=vld[:, g, half * 512:half * 512 + 512],
                            start=(g == 0), stop=(g == 1))
                    if oc == 0:
                        nc.scalar.copy(y_sb[0:ow, half * 512:half * 512 + 512],
                                       yps[0:ow, :])
                    else:
                        nc.vector.tensor_copy(
                            out=y_sb[0:ow, half * 512:half * 512 + 512],
                            in_=yps[0:ow, :])
                eng = (nc.sync, nc.scalar, nc.gpsimd)[(pg * 2 + oc) % 3]
                eng.dma_start(o_flat[obase:obase + ow, px:px + 1024],
                              y_sb[0:ow, :])

    def body(_it=None):
        prev = None
        for i in range(NT):
            qkT = pass1_tile(i, (prev, i - 1) if (prev is not None and
                                                  PHASES >= 3) else None)
            prev = qkT
        if PHASES >= 4:
            # norm prep + v prefetch overlap the last tile's transposes
            rnq8, rnk_bc = pass2_normprep()
            vlds = [load_v(0), load_v(1)]
        if PHASES >= 3 and prev is not None:
            # epilogue gram for the last tile
            for blk in range(16):
                for g in range(2):
                    nc.tensor.matmul(
                        Gps[:, g, :],
                        lhsT=prev[:, blk, g * 96:g * 96 + 96],
                        rhs=prev[:, blk, C + g * 96:C + g * 96 + 96],
                        start=False, stop=(blk == 15),
                        skip_group_check=True)
        if PHASES >= 4:
            MT = pass2_attn(rnq8, rnk_bc)
            pass2_y(MT, vlds)

    if repeat > 1:
        with tc.For_i(0, repeat, 1) as it:
            body(it)
    else:
        body()

    ctx.close()


_CACHE = {}


def _get_runner(repeat=None):
    key = ("runner", repeat)
    if key in _CACHE:
        return _CACHE[key]

    import jax
    from jax.sharding import Mesh, PartitionSpec
    from jax.experimental.shard_map import shard_map
    from concourse import mybir
    from concourse import bass2jax

    nc = build_kernel(repeat=repeat)
    bass2jax.install_neuronx_cc_hook()

    partition_name = (nc.partition_id_tensor.name
                      if nc.partition_id_tensor else None)
    in_names, out_names, out_avals, zero_shapes = [], [], [], []
    for alloc in nc.m.functions[0].allocations:
        if not isinstance(alloc, mybir.MemoryLocationSet):
            continue
        name = alloc.memorylocations[0].name
        if alloc.kind == "ExternalInput":
            if name != partition_name:
                in_names.append(name)
        elif alloc.kind == "ExternalOutput":
            out_names.append(name)
            shape = tuple(alloc.tensor_shape)
            dtype = mybir.dt.np(alloc.dtype)
            out_avals.append(jax.core.ShapedArray(shape, dtype))
            zero_shapes.append((shape, dtype))
    n_params = len(in_names)
    all_names = in_names + out_names
    if partition_name is not None:
        all_names = all_names + [partition_name]

    def _body(*args):
        operands = list(args)
        if partition_name is not None:
            operands.append(bass2jax.partition_id_tensor())
        outs = bass2jax._bass_exec_p.bind(
            *operands,
            out_avals=tuple(out_avals),
            in_names=tuple(all_names),
            out_names=tuple(out_names),
            lowering_input_output_aliases=(),
            sim_require_finite=True,
            sim_require_nnan=True,
            nc=nc,
        )
        return tuple(outs)

    devices = jax.devices()[:B]
    mesh = Mesh(np.asarray(devices), ("core",))
    n_outs = len(out_names)
    sharded = jax.jit(
        shard_map(_body, mesh=mesh,
                  in_specs=(PartitionSpec("core"),) * (n_params + n_outs),
                  out_specs=(PartitionSpec("core"),) * n_outs,
                  check_rep=False),
        donate_argnums=tuple(range(n_params, n_params + n_outs)),
        keep_unused=True,
    )
    runner = (sharded, in_names, out_names, zero_shapes, mesh)
    _CACHE[key] = runner
    return runner


def _prep_inputs(inputs):
    x = np.ascontiguousarray(np.asarray(inputs["x"], dtype=np.float32))
    per_core = {
        "x": x,  # (B, C, H, W): axis0 is already the shard axis
        "w_qkv": np.tile(np.asarray(inputs["w_qkv"], np.float32)[None], (B, 1, 1)),
        "w_dw": np.tile(np.asarray(inputs["w_dw"], np.float32)[None], (B, 1, 1, 1, 1)),
        "w_proj": np.tile(np.asarray(inputs["w_proj"], np.float32)[None], (B, 1, 1)),
        "temperature": np.tile(np.asarray(inputs["temperature"], np.float32)[None],
                               (B, 1, 1, 1)),
    }
    # concat along axis 0: each core's shard must equal the BIR per-core shape
    return {k: v.reshape((-1,) + v.shape[2:]) for k, v in per_core.items()}


def kernel(**inputs) -> np.ndarray:
    sharded, in_names, out_names, zero_shapes, mesh = _get_runner()
    flat = _prep_inputs(inputs)
    args = [flat[name] for name in in_names]
    zeros = [np.zeros((B * s[0],) + tuple(s[1:]), dt) for s, dt in zero_shapes]
    outs = sharded(*args, *zeros)
    y = np.asarray(outs[out_names.index("out")])
    return y.reshape(B, C, H, W).astype(np.float32)


if __name__ == "__main__":
    rng = np.random.default_rng(0)
    demo = {
        "x": rng.standard_normal((B, C, H, W), dtype=np.float32),
        "w_qkv": rng.standard_normal((C3, C), dtype=np.float32) / np.sqrt(C),
        "w_dw": rng.standard_normal((C3, 1, 3, 3), dtype=np.float32) / 3.0,
        "w_proj": rng.standard_normal((C, C), dtype=np.float32) / np.sqrt(C),
        "temperature": np.ones((NH, 1, 1), np.float32),
    }
    out = kernel(**demo)
    print(out.shape, out.dtype, np.abs(out).mean())


# revision 4
# speedup vs baseline: 1.8072x; 1.8072x over previous
"""MDTA (Restormer multi-dconv-head transposed attention) Trainium2 kernel, v2.

Distribution: data-parallel over batch B=8 across 8 NeuronCores (one image
per core, weights replicated, no collectives).

Per-core pipeline (image = 192ch x 128x128, fp32 in/out):
  1. x loaded fp16 via gpsimd cast-DMA
  2. 1x1 qkv conv     : PE matmul fp16 -> psum -> u fp16 [128,5,18,132]
                        chunks {q128, q64|k64, k128, v96, v96}
  3. depthwise 3x3    : q,k chunks on DVE (6 aligned taps 2x-mode) with ACT
                        computing the 3 center-column taps into tmp + DVE add;
                        v chunks on PE (diag matmuls) -> v_sb [96,2,N] fp16
  4. q,k              : spilled to DRAM fp16, read back via xbar DMA-transpose
                        [384,128]->[128,384], split across sync+scalar rings
     gram S_h=q_h k_h^T: per-head 48x48, PSUM-accumulated over all 16384 px,
                        pipelined one tile behind the transposes
     norms            : ACT Square with accum_out per tile
  5. attn finalize    : scale by 1/||q||/||k||*temp, softmax -> attn_g [96,96]
     M^T = attn^T @ Wp^T (2 matmuls) -> y = M^T.T @ v : PE, PSUM -> DRAM direct
"""

import os
import numpy as np

# Hardcoded problem shape (nn_MDTA_74045236183622)
B = 8
C = 192
C3 = 3 * C  # 576
H = W = 128
NPIX = H * W  # 16384
NH, DH = 4, 48
EPS = 1e-12

ROWS = 16                 # output rows per spatial tile
NT = H // ROWS            # 8 tiles
TPX = ROWS * W            # 2048 px per tile
WP = W + 4                # x-padded row width in u (132 -> 264B rows, 4B align)

# u channel chunks over the 576 qkv channels
CHUNK_OFF = [0, 128, 256, 384, 480]
CHUNK_W = [128, 128, 128, 96, 96]
# dw-conv rows owned by PE per chunk (from row 0); the rest go to DVE
DW_PE_ROWS = [int(v) for v in os.environ.get(
    "MDTA_DW_PE", "16,16,16,16,16").split(",")]
GRAM_LAG = int(os.environ.get("MDTA_GRAM_LAG", "3"))
DW_TAP_OUTER = os.environ.get("MDTA_TAP_OUTER", "0") == "1"
PSB_BUFS = int(os.environ.get("MDTA_PSB", "3"))
PSA_BUFS = int(os.environ.get("MDTA_PSA", "2"))
DVE_MODE = os.environ.get("MDTA_DVE_MODE",
                          "pure" if os.environ.get("MDTA_DVE_PURE") == "1"
                          else "pool")  # pure | duo | pool
REPEAT = 1
PHASES = int(os.environ.get("MDTA_PHASES", "4"))  # 1=qkv 2=+dw 3=+gram 4=full

# taps ordered: aligned (dx=+-1) first, center column (dx=0) last
TAPS_ALIGNED = [(dy, dx) for dy in (-1, 0, 1) for dx in (-1, 1)]
TAPS_CENTER = [(dy, 0) for dy in (-1, 0, 1)]


def build_kernel(repeat=None):
    import concourse.bass as bass
    import concourse.tile as tile
    from concourse import bacc, mybir
    from concourse.masks import make_identity

    f32 = mybir.dt.float32
    f16 = mybir.dt.float16

    nc = bacc.Bacc("TRN2", target_bir_lowering=False, debug=False,
                   enable_asserts=False, num_devices=1)

    x_d = nc.dram_tensor("x", (C, H, W), f32, kind="ExternalInput").ap()
    wqkv_d = nc.dram_tensor("w_qkv", (C3, C), f32, kind="ExternalInput").ap()
    wdw_d = nc.dram_tensor("w_dw", (C3, 1, 3, 3), f32, kind="ExternalInput").ap()
    wproj_d = nc.dram_tensor("w_proj", (C, C), f32, kind="ExternalInput").ap()
    temp_d = nc.dram_tensor("temperature", (NH, 1, 1), f32, kind="ExternalInput").ap()
    out_d = nc.dram_tensor("out", (C, H, W), f32, kind="ExternalOutput").ap()

    with tile.TileContext(nc) as tc:
        _emit(tc, bass, mybir, make_identity, f32, f16,
              x_d, wqkv_d, wdw_d, wproj_d, temp_d, out_d,
              repeat if repeat is not None else REPEAT)
    nc.compile()
    return nc


def _emit(tc, bass, mybir, make_identity, f32, f16,
          x_d, wqkv_d, wdw_d, wproj_d, temp_d, out_d, repeat):
    from contextlib import ExitStack
    ctx = ExitStack()
    nc = tc.nc
    Alu = mybir.AluOpType
    Act = mybir.ActivationFunctionType

    persist = ctx.enter_context(tc.tile_pool(name="persist", bufs=1))
    xpool = ctx.enter_context(tc.tile_pool(name="xpool", bufs=2))
    upool = ctx.enter_context(tc.tile_pool(name="upool", bufs=1))
    stpool = ctx.enter_context(tc.tile_pool(name="stage", bufs=2))
    qkpool = ctx.enter_context(tc.tile_pool(name="qkT", bufs=1 + GRAM_LAG))
    tmpool = ctx.enter_context(tc.tile_pool(name="tmul", bufs=4))
    vstpool = ctx.enter_context(tc.tile_pool(name="vstage", bufs=1))
    vldpool = ctx.enter_context(tc.tile_pool(name="vload", bufs=3))
    dram = ctx.enter_context(tc.tile_pool(name="dram", bufs=1, space="DRAM"))
    psA = ctx.enter_context(tc.tile_pool(name="psA", bufs=PSA_BUFS, space="PSUM"))
    psB = ctx.enter_context(tc.tile_pool(name="psB", bufs=PSB_BUFS, space="PSUM"))
    psG = ctx.enter_context(tc.tile_pool(name="psG", bufs=1, space="PSUM"))

    # ---------------- setup: weights into SBUF ----------------
    ident128h = persist.tile([128, 128], f16)
    make_identity(nc, ident128h)
    ident96h = persist.tile([96, 96], f16)
    make_identity(nc, ident96h)

    # natural (contiguous) weight loads, cast fp16, then on-chip PE transposes
    wq_nat = xpool.tile([128, 5, C], f32, name="wq_nat", tag="xh")
    nc.sync.dma_start(wq_nat[:, 0:4, :],
                      wqkv_d[0:512].rearrange("(ci p) c -> p ci c", p=128))
    nc.sync.dma_start(wq_nat[0:64, 4, :], wqkv_d[512:576])
    wq_h = stpool.tile([128, 5, C], f16, name="wq_h", tag="stage")
    nc.vector.tensor_copy(out=wq_h[:, 0:4, :], in_=wq_nat[:, 0:4, :])
    nc.vector.tensor_copy(out=wq_h[0:64, 4, :], in_=wq_nat[0:64, 4, :])

    # w_qkv^T as lhsT: [c_part, o]; K split 128+64  (o chunking done at use)
    wqkvT_a = persist.tile([128, C3], f16)
    wqkvT_b = persist.tile([64, C3], f16)
    for ci in range(5):
        m = 128 if ci < 4 else 64
        o0 = ci * 128
        for kc, kw in ((0, 128), (1, 64)):
            wtp = psA.tile([128, 512], f16, tag="psA", name="wtp")
            nc.tensor.transpose(wtp[0:kw, 0:m],
                                wq_h[0:m, ci, kc * 128:kc * 128 + kw],
                                ident128h[0:m, 0:m])
            dst = wqkvT_a if kc == 0 else wqkvT_b
            nc.scalar.copy(dst[0:kw, o0:o0 + m], wtp[0:kw, 0:m])

    # depthwise weights: wdw_sb[p, ci, t] = w_dw[CHUNK_OFF[ci]+p, 0, t//3, t%3]
    wdw_sb = persist.tile([128, 5, 9], f32)
    wdw_flat = wdw_d.rearrange("o one ky kx -> o (one ky kx)")  # (576, 9)
    with nc.allow_non_contiguous_dma(reason="one-time dw weight load"):
        nc.sync.dma_start(
            wdw_sb[:, 0:3, :],
            wdw_flat[0:384].rearrange("(ci p) t -> p ci t", p=128))
        nc.sync.dma_start(wdw_sb[0:96, 3, :], wdw_flat[384:480])
        nc.sync.dma_start(wdw_sb[0:96, 4, :], wdw_flat[480:576])

    # diag(w_dw) blocks for the PE depthwise path, fp16
    pe_chunks = [ci for ci in range(5) if DW_PE_ROWS[ci] > 0]
    diag_sb = persist.tile([128, 9, len(pe_chunks), 128], f16)
    for t in range(9):
        for k, ci in enumerate(pe_chunks):
            m = CHUNK_W[ci]
            nc.vector.tensor_scalar_mul(
                diag_sb[0:m, t, k, 0:m], ident128h[0:m, 0:m],
                wdw_sb[0:m, ci, t:t + 1])
    diag_idx = {ci: k for k, ci in enumerate(pe_chunks)}

    # w_proj^T as lhsT: [d_part(96), g, o] fp16
    wp_nat = xpool.tile([96, 2, C], f32, name="wp_nat", tag="xh")
    nc.sync.dma_start(wp_nat, wproj_d.rearrange("(ko p) c -> p ko c", p=96))
    wp_h = stpool.tile([96, 2, C], f16, name="wp_h", tag="stage")
    nc.vector.tensor_copy(out=wp_h, in_=wp_nat)
    wpT = persist.tile([96, 2, C], f16)
    for ko in range(2):
        for kc in range(2):
            wtp2 = psA.tile([96, 96], f16, tag="psA", name="wtp2")
            nc.tensor.transpose(wtp2, wp_h[:, ko, kc * 96:kc * 96 + 96],
                                ident96h)
            nc.scalar.copy(wpT[:, kc, ko * 96:ko * 96 + 96], wtp2)

    # persistent working buffers
    u_t = [upool.tile([128, 5, ROWS + 2, WP], f16, name=f"u{j}")
           for j in range(2)]
    for j in range(2):
        nc.vector.memset(u_t[j][:, :, :, 0:1], 0.0)      # left pad col
        nc.vector.memset(u_t[j][:, :, :, W + 1:WP], 0.0)  # right pad cols
    np_part = persist.tile([128, 3, NT], f32)             # per-tile sum-of-squares
    v_dram = dram.tile([2, 96, NPIX], f16)                # v spill
    qk_dram = dram.tile([384, NPIX], f16)                 # q,k spill for transposes
    qk_w = qk_dram.rearrange("(ci p) n -> p ci n", p=128)

    Gps = psG.tile([96, 2, 96], f32, tag="psG", name="Gps")  # gram accum (2-head packed)

    TAPS9 = TAPS_ALIGNED + TAPS_CENTER

    # ---------------- pass 1: one spatial tile ----------------
    def pass1_tile(i, do_gram):
        y0 = i * ROWS
        u = u_t[i % 2]

        # x rows y0-1 .. y0+16 -> xh rows 0..17 (fp16 cast in DMA)
        xh = xpool.tile([128, 2, ROWS + 2, W], f16, name="xh")
        lo = max(y0 - 1, 0)
        hi = min(y0 + ROWS + 1, H)
        ur0 = lo - (y0 - 1)
        if i == 0:
            nc.vector.memset(xh[:, :, 0:1, :], 0.0)
        if i == NT - 1:
            nc.vector.memset(xh[:, :, ROWS + 1:ROWS + 2, :], 0.0)
        nc.gpsimd.dma_start(xh[:, 0, ur0:ur0 + (hi - lo), :], x_d[0:128, lo:hi, :])
        nc.gpsimd.dma_start(xh[0:64, 1, ur0:ur0 + (hi - lo), :],
                            x_d[128:192, lo:hi, :])

        # ---- 1x1 qkv conv: u rows 0..17 ----
        rgroups = [(0, 4), (4, 8), (8, 12), (12, 16), (16, 18)]
        for ci in range(5):
            m = CHUNK_W[ci]
            o0 = CHUNK_OFF[ci]
            for gi, (r0, r1) in enumerate(rgroups):
                n = (r1 - r0) * W
                ps = psA.tile([128, 512], f32, tag="psA", name="ups")
                nc.tensor.matmul(
                    ps[0:m, 0:n], lhsT=wqkvT_a[:, o0:o0 + m],
                    rhs=xh[:, 0, r0:r1, :], start=True, stop=False)
                nc.tensor.matmul(
                    ps[0:m, 0:n], lhsT=wqkvT_b[:, o0:o0 + m],
                    rhs=xh[0:64, 1, r0:r1, :], start=False, stop=True)
                udst = u[0:m, ci, r0:r1, 1:W + 1]
                usrc = ps[0:m, 0:n].rearrange("p (r c) -> p r c", c=W)
                if (gi + ci) % 2 == 0:
                    nc.scalar.copy(udst, usrc)
                else:
                    nc.vector.tensor_copy(out=udst, in_=usrc)

        # ---- gram for previous tile (transposes have had a tile to land) ----
        if do_gram is not None:
            qkT_p, ip = do_gram
            for blk in range(16):
                for g in range(2):
                    nc.tensor.matmul(
                        Gps[:, g, :],
                        lhsT=qkT_p[:, blk, g * 96:g * 96 + 96],
                        rhs=qkT_p[:, blk, C + g * 96:C + g * 96 + 96],
                        start=(ip == 0 and blk == 0),
                        stop=(ip == NT - 1 and blk == 15),
                        skip_group_check=True)

        if PHASES < 2:
            return None
        # ---- depthwise 3x3 ----
        stage = stpool.tile([128, 3, TPX], f16, name="stage")
        vst = vstpool.tile([96, 2, TPX], f16, name="vst")

        def dw_pe(ci, ov, m, r0, r1):
            k = diag_idx[ci]
            if DW_TAP_OUTER:
                # tap-outer: consecutive matmuls share the stationary diag
                pss = [psB.tile([128, 512], f32, tag="psB", name="dps")
                       for _ in range((r1 - r0) // 4)]
                for t, (dy, dx) in enumerate(TAPS9):
                    tcol = 3 * (dy + 1) + dx + 1
                    for pi, oy in enumerate(range(r0, r1, 4)):
                        nc.tensor.matmul(
                            pss[pi][0:m, :],
                            lhsT=diag_sb[0:m, tcol, k, 0:m],
                            rhs=u[0:m, ci, oy + dy + 1:oy + dy + 5,
                                  dx + 1:dx + 1 + W],
                            start=(t == 0), stop=(t == 8),
                            skip_group_check=True)
                for pi, oy in enumerate(range(r0, r1, 4)):
                    dst = ov[:, oy:oy + 4, :]
                    src = pss[pi][0:m, :].rearrange("p (r c) -> p r c", c=W)
                    if (oy // 4 + ci) % 2 == 0:
                        nc.scalar.copy(dst, src)
                    else:
                        nc.vector.tensor_copy(out=dst, in_=src)
                return
            for oy in range(r0, r1, 4):  # out rows oy..oy+4
                ps = psB.tile([128, 512], f32, tag="psB", name="dps")
                for t, (dy, dx) in enumerate(TAPS9):
                    tcol = 3 * (dy + 1) + dx + 1
                    nc.tensor.matmul(
                        ps[0:m, :],
                        lhsT=diag_sb[0:m, tcol, k, 0:m],
                        rhs=u[0:m, ci, oy + dy + 1:oy + dy + 5,
                              dx + 1:dx + 1 + W],
                        start=(t == 0), stop=(t == 8))
                dst = ov[:, oy:oy + 4, :]
                src = ps[0:m, :].rearrange("p (r c) -> p r c", c=W)
                if (oy // 4 + ci) % 2 == 0:
                    nc.scalar.copy(dst, src)
                else:
                    nc.vector.tensor_copy(out=dst, in_=src)

        def dw_dve_pure(ci, ov, m, r0, r1):
            # single-engine DVE chain: TS 4x mults + TT 2x adds, no Pool/ACT
            rows = r1 - r0
            wcol = wdw_sb[0:m, ci, :]
            ovs = ov[:, r0:r1, :]

            def shift(dy, dx):
                return u[0:m, ci, r0 + dy + 1:r0 + dy + 1 + rows,
                         dx + 1:dx + 1 + W]

            def wc(dy, dx):
                tcol = 3 * (dy + 1) + dx + 1
                return wcol[:, tcol:tcol + 1]

            taps = TAPS_ALIGNED + TAPS_CENTER
            dy0, dx0 = taps[0]
            nc.vector.tensor_scalar_mul(ovs, shift(dy0, dx0), wc(dy0, dx0))
            for (dy, dx) in taps[1:]:
                tv = tmpool.tile([128, ROWS, W], f16, name="tv", tag="tm")
                nc.vector.tensor_scalar_mul(tv[0:m, 0:rows], shift(dy, dx),
                                            wc(dy, dx))
                nc.vector.tensor_tensor(ovs, ovs, tv[0:m, 0:rows], op=Alu.add)

        def dw_dve_duo(ci, ov, m, r0, r1):
            # DVE taps + ACT center-column mults (DVE adds); no Pool
            rows = r1 - r0
            wcol = wdw_sb[0:m, ci, :]
            ovs = ov[:, r0:r1, :]

            def shift(dy, dx):
                return u[0:m, ci, r0 + dy + 1:r0 + dy + 1 + rows,
                         dx + 1:dx + 1 + W]

            def wc(dy, dx):
                tcol = 3 * (dy + 1) + dx + 1
                return wcol[:, tcol:tcol + 1]

            cms = []
            for (dy, dx) in TAPS_CENTER:
                tmv = tmpool.tile([128, ROWS, W], f16, name="tm", tag="tm")
                nc.scalar.activation(tmv[0:m, 0:rows], shift(dy, dx), Act.Copy,
                                     scale=wc(dy, dx))
                cms.append(tmv)
            dy0, dx0 = TAPS_ALIGNED[0]
            nc.vector.tensor_scalar_mul(ovs, shift(dy0, dx0), wc(dy0, dx0))
            for (dy, dx) in TAPS_ALIGNED[1:]:
                tv = tmpool.tile([128, ROWS, W], f16, name="tv", tag="tm")
                nc.vector.tensor_scalar_mul(tv[0:m, 0:rows], shift(dy, dx),
                                            wc(dy, dx))
                nc.vector.tensor_tensor(ovs, ovs, tv[0:m, 0:rows], op=Alu.add)
            for tmv in cms:
                nc.vector.tensor_tensor(ovs, ovs, tmv[0:m, 0:rows], op=Alu.add)

        def dw_dve(ci, ov, m, r0, r1):
            if DVE_MODE == "pure":
                return dw_dve_pure(ci, ov, m, r0, r1)
            if DVE_MODE == "duo":
                return dw_dve_duo(ci, ov, m, r0, r1)
            # DVE: 5 aligned taps (TS 4x mults + TT 2x adds)
            # Pool: 1 aligned tap + add of one ACT tmp -> pacc
            # ACT: 3 center-column tap mults -> tmps
            rows = r1 - r0
            wcol = wdw_sb[0:m, ci, :]
            ovs = ov[:, r0:r1, :]

            def shift(dy, dx):
                return u[0:m, ci, r0 + dy + 1:r0 + dy + 1 + rows,
                         dx + 1:dx + 1 + W]

            def wc(dy, dx):
                tcol = 3 * (dy + 1) + dx + 1
                return wcol[:, tcol:tcol + 1]

            cm0 = tmpool.tile([128, ROWS, W], f16, name="tm", tag="tm")
            cm1 = tmpool.tile([128, ROWS, W], f16, name="tm", tag="tm")
            cm2 = tmpool.tile([128, ROWS, W], f16, name="tm", tag="tm")
            for tmv, (dy, dx) in zip((cm0, cm1, cm2), TAPS_CENTER):
                nc.scalar.activation(tmv[0:m, 0:rows], shift(dy, dx), Act.Copy,
                                     scale=wc(dy, dx))
            # Pool subtree
            pacc = tmpool.tile([128, ROWS, W], f16, name="pacc", tag="tm")
            dyp, dxp = TAPS_ALIGNED[5]
            nc.gpsimd.tensor_scalar_mul(pacc[0:m, 0:rows], shift(dyp, dxp),
                                        wc(dyp, dxp))
            nc.gpsimd.tensor_tensor(pacc[0:m, 0:rows], pacc[0:m, 0:rows],
                                    cm2[0:m, 0:rows], op=Alu.add)
            # DVE main chain
            dy0, dx0 = TAPS_ALIGNED[0]
            nc.vector.tensor_scalar_mul(ovs, shift(dy0, dx0), wc(dy0, dx0))
            for (dy, dx) in TAPS_ALIGNED[1:5]:
                tv = tmpool.tile([128, ROWS, W], f16, name="tv", tag="tm")
                nc.vector.tensor_scalar_mul(tv[0:m, 0:rows], shift(dy, dx),
                                            wc(dy, dx))
                nc.vector.tensor_tensor(ovs, ovs, tv[0:m, 0:rows], op=Alu.add)
            nc.vector.tensor_tensor(ovs, ovs, cm0[0:m, 0:rows], op=Alu.add)
            nc.vector.tensor_tensor(ovs, ovs, cm1[0:m, 0:rows], op=Alu.add)
            nc.vector.tensor_tensor(ovs, ovs, pacc[0:m, 0:rows], op=Alu.add)

        sl = slice(i * TPX, (i + 1) * TPX)
        for ci in range(3):
            m = CHUNK_W[ci]
            ov = stage[0:m, ci, :].rearrange("p (r c) -> p r c", c=W)
            rpe = DW_PE_ROWS[ci]
            if rpe > 0:
                dw_pe(ci, ov, m, 0, rpe)
            if rpe < ROWS:
                dw_dve(ci, ov, m, rpe, ROWS)

        if PHASES >= 3:
            # ---- q,k -> DRAM + transpose readbacks, before the v chunks ----
            nc.sync.dma_start(qk_w[:, 0:2, sl], stage[:, 0:2, :])
            nc.scalar.dma_start(qk_w[:, 2:3, sl], stage[:, 2:3, :])
            qkT = qkpool.tile([128, 16, 384], f16, name="qkT")
            for blk in range(16):
                n0 = i * TPX + blk * 128
                eng = nc.sync if blk % 2 == 0 else nc.scalar
                eng.dma_start_transpose(qkT[:, blk, :], qk_dram[:, n0:n0 + 128])

        for ci in range(3, 5):
            m = CHUNK_W[ci]
            ov = vst[0:m, ci - 3, :].rearrange("p (r c) -> p r c", c=W)
            rpe = DW_PE_ROWS[ci]
            if rpe > 0:
                dw_pe(ci, ov, m, 0, rpe)
            if rpe < ROWS:
                dw_dve(ci, ov, m, rpe, ROWS)

        if PHASES < 3:
            return None
        # ---- norms partial: ACT square with accumulate ----
        # (in-place square after the spill DMAs have read the stage)
        for ci in range(3):
            nc.scalar.activation(
                stage[:, ci, :], stage[:, ci, :], Act.Square,
                accum_out=np_part[:, ci, i:i + 1])

        # ---- v -> DRAM ----
        nc.gpsimd.dma_start(v_dram[:, :, sl].rearrange("g p n -> p g n"), vst)
        return qkT

    # ---------------- pass 2: finalize attention + output ----------------
    def pass2_normprep():
        rn = persist.tile([128, 3], f32, name="rn")
        nc.vector.tensor_reduce(rn, np_part, axis=mybir.AxisListType.X,
                                op=Alu.add)
        nc.scalar.sqrt(rn, rn)
        nc.vector.tensor_scalar_max(rn, rn, EPS)
        nc.vector.reciprocal(rn, rn)

        nrm_dram = dram.tile([128, 3], f32, name="nrm_dram")
        nc.sync.dma_start(nrm_dram, rn)
        # rnq_e/rnq_o[p, g] = 1/||q|| for heads 2g / 2g+1 (q ch c -> rn[c%128, c//128])
        rnq_e = persist.tile([48, 2], f32, name="rnq_e")
        rnq_o = persist.tile([48, 2], f32, name="rnq_o")
        nc.sync.dma_start(rnq_e[:, 0:1], nrm_dram[0:48, 0:1])
        nc.sync.dma_start(rnq_e[0:32, 1:2], nrm_dram[96:128, 0:1])
        nc.sync.dma_start(rnq_e[32:48, 1:2], nrm_dram[0:16, 1:2])
        nc.sync.dma_start(rnq_o[:, 0:1], nrm_dram[48:96, 0:1])
        nc.sync.dma_start(rnq_o[:, 1:2], nrm_dram[16:64, 1:2])
        # temperature: tg_e[p, g] = temp[2g], tg_o[p, g] = temp[2g+1]
        tg = persist.tile([48, 2, 2], f32, name="tg")
        nc.gpsimd.dma_start(
            tg[:, 0, :], bass.AP(tensor=temp_d.tensor, offset=temp_d.offset,
                                 ap=[[0, 48], [2, 2]]))
        nc.gpsimd.dma_start(
            tg[:, 1, :], bass.AP(tensor=temp_d.tensor,
                                 offset=temp_d.offset + 1,
                                 ap=[[0, 48], [2, 2]]))
        nc.vector.tensor_mul(rnq_e, rnq_e, tg[:, 0, :])
        nc.vector.tensor_mul(rnq_o, rnq_o, tg[:, 1, :])

        rnk_row = persist.tile([1, 192], f32, name="rnk_row")
        with nc.allow_non_contiguous_dma(reason="tiny norm vector transpose"):
            nc.sync.dma_start(rnk_row[0:1, 0:64],
                              nrm_dram[64:128, 1:2].rearrange("p o -> o p"))
            nc.sync.dma_start(rnk_row[0:1, 64:192],
                              nrm_dram[0:128, 2:3].rearrange("p o -> o p"))
        ones_row = persist.tile([1, 96], f32, name="ones_row")
        nc.vector.memset(ones_row, 1.0)
        rnk_bc = persist.tile([96, 2, 96], f32, name="rnk_bc")
        for g in range(2):
            bc_ps = psA.tile([128, 512], f32, tag="psA", name="bc_ps")
            nc.tensor.matmul(bc_ps[0:96, 0:96], lhsT=ones_row,
                             rhs=rnk_row[0:1, g * 96:g * 96 + 96],
                             start=True, stop=True)
            nc.vector.tensor_copy(out=rnk_bc[:, g, :], in_=bc_ps[0:96, 0:96])
        return (rnq_e, rnq_o), rnk_bc

    def pass2_attn(rnq8, rnk_bc):
        rnq_e, rnq_o = rnq8
        # scale + softmax per head, assemble block-diag attn groups [96, 96]
        # (odd heads sit at partition offset 48 in the packed gram: realign
        #  their 48x48 blocks to partition 0 via SBUF-SBUF DMA, softmax at
        #  base 0, then DMA the result back to offset 48)
        Sg = persist.tile([96, 2, 96], f32, name="Sg")
        nc.vector.tensor_copy(out=Sg, in_=Gps)
        So = persist.tile([48, 2, DH], f32, name="So")
        for g in range(2):
            nc.sync.dma_start(So[:, g, :], Sg[48:96, g, 48:96])
        attn_g = [persist.tile([96, 96], f16, name=f"attn_g{g}")
                  for g in range(2)]
        for g in range(2):
            nc.vector.memset(attn_g[g], 0.0)
        mx = persist.tile([48, 1], f32, name="mx")
        sm = persist.tile([48, 1], f32, name="sm")
        at16 = persist.tile([48, DH], f16, name="at16")
        for h in range(NH):
            g, odd = h // 2, h % 2
            r0 = odd * DH
            blkS = So[:, g, :] if odd else Sg[0:DH, g, 0:DH]
            rnq = rnq_o if odd else rnq_e
            nc.vector.scalar_tensor_tensor(
                blkS, blkS, rnq[:, g:g + 1],
                rnk_bc[0:DH, g, r0:r0 + DH],
                op0=Alu.mult, op1=Alu.mult)
            nc.vector.tensor_reduce(mx, blkS, axis=mybir.AxisListType.X,
                                    op=Alu.max, negate=True)
            nc.scalar.activation(blkS, blkS, Act.Exp, bias=mx, scale=1.0)
            nc.vector.tensor_reduce(sm, blkS, axis=mybir.AxisListType.X,
                                    op=Alu.add)
            nc.vector.reciprocal(sm, sm)
            if odd:
                nc.vector.tensor_scalar_mul(at16, blkS, sm)
                nc.scalar.dma_start(attn_g[g][48:96, 48:96], at16)
            else:
                nc.vector.tensor_scalar_mul(attn_g[g][0:DH, 0:DH], blkS, sm)

        # M^T[e, o] = sum_d attn[d, e] WpT[d, o]  (block-diag per 96-group)
        MT = persist.tile([96, 2, C], f16, name="MT")
        for g in range(2):
            mps = psA.tile([128, 512], f32, tag="psA", name="mps")
            nc.tensor.matmul(mps[0:96, 0:C], lhsT=attn_g[g], rhs=wpT[:, g, :],
                             start=True, stop=True)
            nc.vector.tensor_copy(out=MT[:, g, :], in_=mps[0:96, 0:C])

        return MT

    def load_v(pg):
        px = pg * 1024
        vld = vldpool.tile([96, 2, 1024], f16, name="vld", tag="vld")
        nc.gpsimd.dma_start(
            vld, v_dram[:, :, px:px + 1024].rearrange("g p n -> p g n"))
        return vld

    def pass2_y(MT, vlds):
        # y = M @ v : PSUM -> SBUF -> DRAM (1024-px groups)
        o_flat = out_d.rearrange("c h w -> c (h w)")
        npg = NPIX // 1024
        for pg in range(npg):
            px = pg * 1024
            vld = vlds[pg]
            if pg + 2 < npg:
                vlds.append(load_v(pg + 2))
            for oc, (obase, ow) in enumerate(((0, 128), (128, 64))):
                y_sb = vldpool.tile([128, 1024], f32, name="y_sb", tag="ysb")
                for half in range(2):
                    yps = psA.tile([128, 512], f32, tag="psA", name="yps")
                    for g in range(2):
                        nc.tensor.matmul(
                            yps[0:ow, :], lhsT=MT[:, g, obase:obase + ow],
                            rhs=vld[:, g, half * 512:half * 512 + 512],
                            start=(g == 0), stop=(g == 1))
                    if oc == 0:
                        nc.scalar.copy(y_sb[0:ow, half * 512:half * 512 + 512],
                                       yps[0:ow, :])
                    else:
                        nc.vector.tensor_copy(
                            out=y_sb[0:ow, half * 512:half * 512 + 512],
                            in_=yps[0:ow, :])
                eng = (nc.sync, nc.scalar, nc.gpsimd)[(pg * 2 + oc) % 3]
                eng.dma_start(o_flat[obase:obase + ow, px:px + 1024],
                              y_sb[0:ow, :])

    def epilogue_gram(qkT_p, ip):
        for blk in range(16):
            for g in range(2):
                nc.tensor.matmul(
                    Gps[:, g, :],
                    lhsT=qkT_p[:, blk, g * 96:g * 96 + 96],
                    rhs=qkT_p[:, blk, C + g * 96:C + g * 96 + 96],
                    start=(ip == 0 and blk == 0),
                    stop=(ip == NT - 1 and blk == 15),
                    skip_group_check=True)

    def body(_it=None):
        hist = []
        for i in range(NT):
            lag = None
            if PHASES >= 3 and len(hist) >= GRAM_LAG:
                lag = (hist[i - GRAM_LAG], i - GRAM_LAG)
            qkT = pass1_tile(i, lag)
            hist.append(qkT)
        if PHASES >= 4:
            # norm prep + v prefetch overlap the last tile's transposes
            rnq8, rnk_bc = pass2_normprep()
            vlds = [load_v(0), load_v(1)]
        if PHASES >= 3 and hist[-1] is not None:
            for ip in range(NT - GRAM_LAG, NT):
                epilogue_gram(hist[ip], ip)
        if PHASES >= 4:
            MT = pass2_attn(rnq8, rnk_bc)
            pass2_y(MT, vlds)

    if repeat > 1:
        with tc.For_i(0, repeat, 1) as it:
            body(it)
    else:
        body()

    ctx.close()


_CACHE = {}


def _get_runner(repeat=None):
    key = ("runner", repeat)
    if key in _CACHE:
        return _CACHE[key]

    import jax
    from jax.sharding import Mesh, PartitionSpec
    from jax.experimental.shard_map import shard_map
    from concourse import mybir
    from concourse import bass2jax

    nc = build_kernel(repeat=repeat)
    bass2jax.install_neuronx_cc_hook()

    partition_name = (nc.partition_id_tensor.name
                      if nc.partition_id_tensor else None)
    in_names, out_names, out_avals, zero_shapes = [], [], [], []
    for alloc in nc.m.functions[0].allocations:
        if not isinstance(alloc, mybir.MemoryLocationSet):
            continue
        name = alloc.memorylocations[0].name
        if alloc.kind == "ExternalInput":
            if name != partition_name:
                in_names.append(name)
        elif alloc.kind == "ExternalOutput":
            out_names.append(name)
            shape = tuple(alloc.tensor_shape)
            dtype = mybir.dt.np(alloc.dtype)
            out_avals.append(jax.core.ShapedArray(shape, dtype))
            zero_shapes.append((shape, dtype))
    n_params = len(in_names)
    all_names = in_names + out_names
    if partition_name is not None:
        all_names = all_names + [partition_name]

    def _body(*args):
        operands = list(args)
        if partition_name is not None:
            operands.append(bass2jax.partition_id_tensor())
        outs = bass2jax._bass_exec_p.bind(
            *operands,
            out_avals=tuple(out_avals),
            in_names=tuple(all_names),
            out_names=tuple(out_names),
            lowering_input_output_aliases=(),
            sim_require_finite=True,
            sim_require_nnan=True,
            nc=nc,
        )
        return tuple(outs)

    devices = jax.devices()[:B]
    mesh = Mesh(np.asarray(devices), ("core",))
    n_outs = len(out_names)
    sharded = jax.jit(
        shard_map(_body, mesh=mesh,
                  in_specs=(PartitionSpec("core"),) * (n_params + n_outs),
                  out_specs=(PartitionSpec("core"),) * n_outs,
                  check_rep=False),
        donate_argnums=tuple(range(n_params, n_params + n_outs)),
        keep_unused=True,
    )
    runner = (sharded, in_names, out_names, zero_shapes, mesh)
    _CACHE[key] = runner
    return runner


def _prep_inputs(inputs):
    x = np.ascontiguousarray(np.asarray(inputs["x"], dtype=np.float32))
    per_core = {
        "x": x,  # (B, C, H, W): axis0 is already the shard axis
        "w_qkv": np.tile(np.asarray(inputs["w_qkv"], np.float32)[None], (B, 1, 1)),
        "w_dw": np.tile(np.asarray(inputs["w_dw"], np.float32)[None], (B, 1, 1, 1, 1)),
        "w_proj": np.tile(np.asarray(inputs["w_proj"], np.float32)[None], (B, 1, 1)),
        "temperature": np.tile(np.asarray(inputs["temperature"], np.float32)[None],
                               (B, 1, 1, 1)),
    }
    # concat along axis 0: each core's shard must equal the BIR per-core shape
    return {k: v.reshape((-1,) + v.shape[2:]) for k, v in per_core.items()}


def kernel(**inputs) -> np.ndarray:
    sharded, in_names, out_names, zero_shapes, mesh = _get_runner()
    flat = _prep_inputs(inputs)
    args = [flat[name] for name in in_names]
    zeros = [np.zeros((B * s[0],) + tuple(s[1:]), dt) for s, dt in zero_shapes]
    outs = sharded(*args, *zeros)
    y = np.asarray(outs[out_names.index("out")])
    return y.reshape(B, C, H, W).astype(np.float32)


if __name__ == "__main__":
    rng = np.random.default_rng(0)
    demo = {
        "x": rng.standard_normal((B, C, H, W), dtype=np.float32),
        "w_qkv": rng.standard_normal((C3, C), dtype=np.float32) / np.sqrt(C),
        "w_dw": rng.standard_normal((C3, 1, 3, 3), dtype=np.float32) / 3.0,
        "w_proj": rng.standard_normal((C, C), dtype=np.float32) / np.sqrt(C),
        "temperature": np.ones((NH, 1, 1), np.float32),
    }
    out = kernel(**demo)
    print(out.shape, out.dtype, np.abs(out).mean())
